# revision 20
# baseline (speedup 1.0000x reference)
"""Trainium2 Bass kernel for nn_CPCModel (CPC-style NCE loss), v2.

Strategy (8 NeuronCores, full inputs on every core, no collectives):

Leave-one-out softmax pooling collapses algebraically:
    pooled[j] = (T - e_j zt_j)/(S - e_j),  e = exp(s), S = sum e, T = sum e zt
so the [B,B] pooling matrix is never materialized.  The loss needs only
    nce = -mean_i( total[i,i] - logsumexp_j total[i,j] )
with  total[i, j in group g] = Azw_g[i]*pooled_g[j] + Czw[i]*c[j] + delta_g[i].

v2 layout choices (all bf16 matmul path, logits pre-scaled by A_SCALE):
 - Two moving tiles, no partition-shift DMA:
     VA [128,2048] = [pooled0 (parts 0:64) ; cT0 (parts 64:128)]  (group-0 cols)
     VB [128,2048] = [cT1 (parts 0:64) ; pooled1 (parts 64:128)]  (group-1 cols)
   with U0 = A*hstack(Ww0,Wk), U1 = A*hstack(Wk,Ww1) matching each K-order.
 - Scores kept in [8,512] layout (row 2*ch+g = chunk ch, group g) so the
   beta chain (e-S, reciprocal) runs on free-size 512 not 2048.
 - ztw STT carries T via free accum_out; GPSIMD does the diagonal
   partition-sum (axis=C) so the diag path needs no PSUM/PE.
 - Main loop: 8 PSUM tiles [128,2048]; most exp'd on ScalarE LUT
   (scale=1/A), some tiles optionally on DVE via Schraudolph int32 bit-trick.
 - Device returns raw row-sums + diag pieces; host does ln + final sum.
"""

import numpy as np

import concourse.bacc as bacc
import concourse.bass as bass
import concourse.mybir as mybir
import concourse.tile as tile
from concourse.bass_utils import run_bass_kernel_spmd

N_CORES = 8
B = 4096
OWN = B // N_CORES            # 512 rows of `total` per core
G = 2048                      # group size
F32 = mybir.dt.float32
BF16 = mybir.dt.bfloat16
I32 = mybir.dt.int32
AF = mybir.ActivationFunctionType
ALU = mybir.AluOpType
AX = mybir.AxisListType
SHIFT = 44.0

# Schraudolph exp: exp(x) ~= bitcast_f32(int32(A_SCALE*x + B_BIAS)).
A_SCALE = float(np.float32(2.0 ** 23 / np.log(2.0)))   # 12102203.16...
B_BIAS = 1064869216.0   # 127*2^23 - 484000, tuned on real logit distribution
INV_A = float(np.float32(1.0 / A_SCALE))

# which of the 8 main tiles use the DVE Schraudolph path (rest: ScalarE LUT)
SCHRAUD_TILES = (2, 5)

# blob column layout (bf16 weight blob wbf [128, WBF]):
#   lwT0 0:64 | lwT1 64:128 | a1wB 128:192 | a2wB 192:194 (rows 0:64)
#   sel2 194:322 (rows 0:2) | b01a 322:324 | uw0p 324:452 | uw1 452:580
WBF = 580


def _build_program():
    nc = bacc.Bacc(
        "TRN2",
        target_bir_lowering=False,
        debug=False,
        num_devices=N_CORES,
    )

    def din(name, shape, dt):
        return nc.dram_tensor(name, shape, dt, kind="ExternalInput").ap()

    zwTb_d = din("zwTb", [128, B], BF16)      # chunk-packed zw.T (see host)
    zwoTb_d = din("zwoTb", [128, OWN], BF16)  # own 512 rows of zw, transposed
    cT0_d = din("cT0", [64, G], BF16)         # c.T cols 0:2048
    cT1_d = din("cT1", [64, G], BF16)         # c.T cols 2048:4096
    wbf_d = din("wbf", [128, WBF], BF16)      # packed small weights
    wf32_d = din("wf32", [128, 2], F32)       # linb2 | a1b2
    out1_d = nc.dram_tensor("out1", [128, 16], F32, kind="ExternalOutput").ap()
    dout_d = nc.dram_tensor("dout", [1, 2 * OWN], F32, kind="ExternalOutput").ap()

    from contextlib import ExitStack
    with tile.TileContext(nc) as tc, ExitStack() as ctx:
        pers = ctx.enter_context(tc.tile_pool(name="pers", bufs=1))
        scr = ctx.enter_context(tc.tile_pool(name="scr", bufs=1))

        # ---------------- DMA loads, split across engine rings --------------
        # small blobs first on the idle SP ring (they gate the U builds);
        # zwTb chunks stream on the Pool ring; cT last (needed only by main).
        wbf = pers.tile([128, WBF], BF16, tag="wbf", name="wbf")
        nc.sync.dma_start(wbf[:], wbf_d[:])
        zwoTb = pers.tile([128, OWN], BF16, tag="zwoTb", name="zwoTb")
        nc.sync.dma_start(zwoTb[:], zwoTb_d[:])
        wf32 = pers.tile([128, 2], F32, tag="wf32", name="wf32")
        nc.sync.dma_start(wf32[:], wf32_d[:])
        zwTb = pers.tile([128, B], BF16, tag="zwTb", name="zwTb")
        for ch in range(4):
            nc.gpsimd.dma_start(zwTb[:, ch * 1024:(ch + 1) * 1024],
                                zwTb_d[:, ch * 1024:(ch + 1) * 1024])

        VA = pers.tile([128, G], BF16, tag="VA", name="VA")
        VB = pers.tile([128, G], BF16, tag="VB", name="VB")
        nc.sync.dma_start(VA[64:128, :], cT0_d[:])
        nc.sync.dma_start(VB[0:64, :], cT1_d[:])

        lwT0 = wbf[:, 0:64]
        lwT1 = wbf[:, 64:128]
        a1wB = wbf[:, 128:192]
        a2wB = wbf[0:64, 192:194]
        sel2 = wbf[0:2, 194:322]
        b01a = wbf[:, 322:324]
        uw0p = wbf[:, 324:452]
        uw1 = wbf[:, 452:580]
        linb2 = wf32[:, 0:1]
        a1b2 = wf32[0:64, 1:2]

        # persistent SBUF state
        out1 = pers.tile([128, 16], F32, tag="out1", name="out1")
        seacc = out1[:, 0:8]
        biasS = out1[:, 8:16]
        dout = pers.tile([1, 2 * OWN], F32, tag="dout", name="dout")
        biasD = pers.tile([128, 8], F32, tag="biasD", name="biasD")
        ztT2 = pers.tile([128, G], BF16, tag="ztT2", name="ztT2")
        hT2 = pers.tile([64, G], BF16, tag="hT2", name="hT2")
        ztwT2 = pers.tile([128, G], BF16, tag="ztwT2", name="ztwT2")
        e2 = pers.tile([2, G], BF16, tag="e2", name="e2")
        Sacc2 = pers.tile([2, 4], F32, tag="Sacc2", name="Sacc2")
        S2 = pers.tile([2, 1], F32, tag="S2", name="S2")
        S2b = pers.tile([2, 1], BF16, tag="S2b", name="S2b")
        Sb = pers.tile([128, 1], F32, tag="Sb", name="Sb")
        Tacc = pers.tile([128, 4], F32, tag="Tacc", name="Tacc")
        T2 = pers.tile([128, 1], F32, tag="T2", name="T2")
        U0 = pers.tile([128, OWN], BF16, tag="U0", name="U0")
        U1 = pers.tile([128, OWN], BF16, tag="U1", name="U1")
        bcs = pers.tile([128, G], BF16, tag="bcs", name="bcs")
        bcs1 = pers.tile([128, G], BF16, tag="bcs1", name="bcs1")
        numer = pers.tile([128, G], BF16, tag="numer", name="numer")
        prodA = pers.tile([128, OWN], F32, tag="prodA", name="prodA")
        prodB = pers.tile([128, OWN], F32, tag="prodB", name="prodB")

        with tc.tile_pool(name="prep", bufs=1, space="PSUM") as prep:
            def ps(name):
                return prep.tile([128, 512], F32, tag="ps", name=name, bufs=3)

            # ---------------- U builds + delta bias columns ----------------
            for uw, U in ((uw0p, U0), (uw1, U1)):
                pu = ps("pu")
                nc.tensor.matmul(pu[:], uw, zwoTb[:], start=True, stop=True)
                nc.scalar.copy(U[:], pu[:])

            pd = prep.tile([128, 8], F32, tag="mi", name="pd")
            for ic in range(4):
                nc.tensor.matmul(pd[:, 2 * ic:2 * ic + 2],
                                 zwoTb[:, ic * 128:(ic + 1) * 128], b01a,
                                 start=True, stop=True)
            # biasS = delta - 44 (unscaled), biasD = A*delta + (B - 44A)
            nc.scalar.activation(biasS, pd[:], AF.Copy, bias=-SHIFT,
                                 scale=INV_A)
            nc.scalar.activation(biasD[:], pd[:], AF.Copy,
                                 bias=B_BIAS - SHIFT * A_SCALE)

            # ---------------- phase 1: score pipeline + ztw ----------------
            for ch in range(4):
                sl = slice(ch * 512, (ch + 1) * 512)
                pz = ps("pz")
                nc.tensor.matmul(pz[0:64, :], lwT0,
                                 zwTb[:, ch * 1024:ch * 1024 + 512],
                                 start=True, stop=True)
                nc.tensor.matmul(pz[64:128, :], lwT1,
                                 zwTb[:, ch * 1024 + 512:ch * 1024 + 1024],
                                 start=True, stop=True)
                nc.vector.tensor_scalar(ztT2[:, sl], pz[:], linb2, 0.0,
                                        op0=ALU.add, op1=ALU.max)
                ph = ps("ph")
                nc.tensor.matmul(ph[0:64, :], a1wB, ztT2[:, sl],
                                 start=True, stop=True)
                nc.scalar.activation(hT2[:, sl], ph[0:64, :], AF.Tanh,
                                     bias=a1b2)
                s2 = prep.tile([2, 512], F32, tag="s2", name="s2", bufs=2)
                nc.tensor.matmul(s2[:], a2wB, hT2[:, sl],
                                 start=True, stop=True)
                nc.scalar.activation(e2[:, sl], s2[:], AF.Exp,
                                     accum_out=Sacc2[:, ch:ch + 1])
                ebc = prep.tile([128, 512], F32, tag="bc", name="ebc", bufs=2)
                nc.tensor.matmul(ebc[:], sel2, e2[:, sl],
                                 start=True, stop=True)
                # ztwT2 holds MINUS zt*e so the pooled numerator (T - ztw)
                # comes out of one subtract; Tacc accumulates -T.
                nc.vector.scalar_tensor_tensor(
                    out=ztwT2[:, sl], in0=ztT2[:, sl], scalar=-1.0,
                    in1=ebc[:], op0=ALU.mult, op1=ALU.mult,
                    accum_out=Tacc[:, ch:ch + 1])

            # ---------------- phase 2: pooled = (T - ztw)/(S - e) ----------
            nc.vector.reduce_sum(T2[:], Tacc[:], axis=AX.X)   # = -T
            nc.vector.reduce_sum(S2[:], Sacc2[:], axis=AX.X)
            nc.vector.tensor_copy(S2b[:], S2[:])
            Sp = prep.tile([128, 1], F32, tag="mi", name="Sp")
            nc.tensor.matmul(Sp[:], sel2, S2b[:], start=True, stop=True)
            nc.vector.tensor_copy(Sb[:], Sp[:])
            for ch in range(4):
                sl = slice(ch * 512, (ch + 1) * 512)
                bbc = prep.tile([128, 512], F32, tag="bc", name="bbc", bufs=2)
                nc.tensor.matmul(bbc[:], sel2, e2[:, sl],
                                 start=True, stop=True)
                # bcs1 = Relu(S - e) == S - e  (strictly positive)
                nc.scalar.activation(bcs1[:, sl], bbc[:], AF.Relu,
                                     scale=-1.0, bias=Sb[:])
                with nc.allow_low_precision(reason="beta bf16"):
                    nc.vector.reciprocal(bcs[:, sl], bcs1[:, sl])
                # numer = -ztw - (-T) = T - ztw;  pooled = numer/(S - e)
                nc.vector.tensor_scalar(numer[:, sl], ztwT2[:, sl],
                                        T2[:], None, op0=ALU.subtract)
                nc.vector.tensor_tensor(VA[0:64, sl], numer[0:64, sl],
                                        bcs[0:64, sl], op=ALU.mult)
                nc.gpsimd.tensor_tensor(VB[64:128, sl], numer[64:128, sl],
                                        bcs[64:128, sl], op=ALU.mult)

        # ---------------- diagonal pieces (no PSUM needed) -----------------
        pid = nc.vector.partition_id()
        vsl = bass.ts(pid % 4, OWN)
        nc.vector.tensor_tensor(prodA[:], U0[:], VA[:, vsl], op=ALU.mult)
        nc.vector.tensor_tensor(prodB[:], U1[:], VB[:, vsl], op=ALU.mult)
        nc.gpsimd.reduce_sum(dout[0:1, 0:OWN], prodA[:], axis=AX.C)
        nc.gpsimd.reduce_sum(dout[0:1, OWN:2 * OWN], prodB[:], axis=AX.C)

        # ---------------- main loop: 8 tiles of [128 rows x 2048 cols] -----
        with tc.tile_pool(name="mainp", bufs=2, space="PSUM") as mainp:
            for t in range(8):
                ic, g = t // 2, t % 2
                U = U0 if g == 0 else U1
                V = VA if g == 0 else VB
                pm = mainp.tile([128, G], F32, tag="pb", name="pm")
                for q in range(4):
                    nc.tensor.matmul(pm[:, q * 512:(q + 1) * 512],
                                     U[:, ic * 128:(ic + 1) * 128],
                                     V[:, q * 512:(q + 1) * 512],
                                     start=True, stop=True)
                if t in SCHRAUD_TILES:
                    esi = scr.tile([128, G], I32, tag="esi", name="esi")
                    nc.vector.tensor_scalar(esi[:], pm[:],
                                            biasD[:, t:t + 1], 0.0,
                                            op0=ALU.add, op1=ALU.max)
                    nc.vector.reduce_sum(seacc[:, t:t + 1],
                                         esi[:].bitcast(F32), axis=AX.X)
                else:
                    es = scr.tile([128, G], BF16, tag="es", name="es")
                    nc.scalar.activation(es[:], pm[:], AF.Exp,
                                         bias=biasS[:, t:t + 1], scale=INV_A,
                                         accum_out=seacc[:, t:t + 1])

        nc.gpsimd.dma_start(out1_d[:], out1[:])
        nc.gpsimd.dma_start(dout_d[:], dout[:])

    nc.compile()
    return nc


_built = None


def _get_program():
    global _built
    if _built is None:
        _built = _build_program()
    return _built


def make_in_maps(inputs):
    import ml_dtypes
    BF = ml_dtypes.bfloat16
    f = lambda x: np.ascontiguousarray(np.asarray(x, dtype=np.float32))
    bf = lambda x: np.ascontiguousarray(np.asarray(x, np.float32).astype(BF))

    zw = np.concatenate([f(inputs['zw_0']), f(inputs['zw_1'])], axis=0)
    zwT = np.ascontiguousarray(zw.T)                      # [128, 4096]
    # chunk-packed: block ch = [g0 cols ch*512:+512 | g1 cols ch*512:+512]
    zwTbCH = np.empty((128, B), np.float32)
    for ch in range(4):
        zwTbCH[:, ch * 1024:ch * 1024 + 512] = zwT[:, ch * 512:(ch + 1) * 512]
        zwTbCH[:, ch * 1024 + 512:ch * 1024 + 1024] = \
            zwT[:, G + ch * 512:G + (ch + 1) * 512]

    wk = f(inputs['Wk_w'])
    b0 = f(inputs['Ww0_b']) + f(inputs['Wk_b'])
    b1 = f(inputs['Ww1_b']) + f(inputs['Wk_b'])

    a1wB = np.zeros((128, 64), np.float32)
    a1wB[0:64, 0:32] = f(inputs['a0_1w']).T
    a1wB[64:128, 32:64] = f(inputs['a1_1w']).T
    a2wB = np.zeros((64, 2), np.float32)
    a2wB[0:32, 0:1] = f(inputs['a0_2w']).T
    a2wB[32:64, 1:2] = f(inputs['a1_2w']).T
    sel2 = np.zeros((2, 128), np.float32)
    sel2[0, 0:64] = 1.0
    sel2[1, 64:128] = 1.0

    wbf = np.zeros((128, WBF), np.float32)
    wbf[:, 0:64] = f(inputs['lin0_w']).T
    wbf[:, 64:128] = f(inputs['lin1_w']).T
    wbf[:, 128:192] = a1wB
    wbf[0:64, 192:194] = a2wB
    wbf[0:2, 194:322] = sel2
    wbf[:, 322:323] = (A_SCALE * b0).reshape(128, 1)
    wbf[:, 323:324] = (A_SCALE * b1).reshape(128, 1)
    wbf[:, 324:452] = A_SCALE * np.hstack([f(inputs['Ww0_w']), wk])
    wbf[:, 452:580] = A_SCALE * np.hstack([wk, f(inputs['Ww1_w'])])

    wf32 = np.zeros((128, 2), np.float32)
    wf32[:, 0] = np.concatenate([f(inputs['lin0_b']), f(inputs['lin1_b'])])
    wf32[0:64, 1] = np.concatenate([f(inputs['a0_1b']), f(inputs['a1_1b'])])

    cT = f(inputs['c']).T                                  # [64, 4096]

    base = {
        'zwTb': bf(zwTbCH),
        'cT0': bf(cT[:, 0:G]),
        'cT1': bf(cT[:, G:B]),
        'wbf': bf(wbf),
        'wf32': wf32,
    }
    in_maps = []
    for cid in range(N_CORES):
        m = dict(base)
        m['zwoTb'] = bf(zwT[:, cid * OWN:(cid + 1) * OWN])
        in_maps.append(m)
    return in_maps


def kernel(**inputs):
    nc = _get_program()
    in_maps = make_in_maps(inputs)
    res = run_bass_kernel_spmd(nc, in_maps, list(range(N_CORES)))
    total = 0.0
    for cid, r in enumerate(res.results):
        g = cid // 4
        out1 = np.asarray(r['out1'], dtype=np.float64)   # [128,16]
        dout = np.asarray(r['dout'], dtype=np.float64).reshape(2, 512)
        seacc, biasS = out1[:, 0:8], out1[:, 8:16]
        diag_raw = dout[g, :] / A_SCALE                  # [512] own rows
        for ic in range(4):
            p = np.arange(128)
            u = ic * 128 + p
            sumexp = seacc[p, 2 * ic] + seacc[p, 2 * ic + 1]
            v = diag_raw[u] + biasS[p, 2 * ic + g] - np.log(sumexp)
            total += v.sum()
    return np.array(-(total / B), dtype=np.float32)
